# revision 53
# baseline (speedup 1.0000x reference)
"""Causal self-attention (B=2, S=2048, D=2048, H=16, Hd=128) on 8 trn2 cores.

Sharding: DP=2 over batch x TP=4 over heads. Core c handles batch b = c//4 and
global heads [4t, 4t+4) with t = c%4. Inputs are sharded/transposed on the
host with numpy; the full output y is assembled on the host from per-core
y^T slices.

Per-core SPMD program -- one software-pipelined loop over 512-col seq chunks,
with attention in the TRANSPOSED (k-major) orientation so no PE transposes of
P are needed (vs the q-major form this removes ~830 small PE instructions and
~100us of DVE work per core):
  - QKV projection (bf16 matmuls, fp32 PSUM accum): chunk n of qT/kT in
    (hd, seq) layout and v in per-head (seq-block, blocks) layout (PE
    transpose), with per-partition bias adds on DVE. Head-major m-order so
    head h's attention starts while later heads project. x and all weights
    load via grouped multi-block DMAs (rearranged access patterns) to cut
    serialized HWDGE descriptor-generation time; next chunk's x prefetches
    ahead of the attention-phase DMAs.
  - Attention for q-chunk n: for every k-block j <= 4n+3, one matmul
    S^T[j, n-cols] = kT_j-stationary @ qT-moving (narrow moving width for
    diagonal-group blocks), exp on ScalarE (no max-subtraction; qk dots over
    128 dims are O(10); fp32 exp cannot overflow) -> P^T slice in SBUF bf16,
    with the diagonal block's causal triangle zeroed in place on the
    otherwise-idle GpSimd engine (off the PE critical path). Softmax denominator l lands broadcast across all partitions
    via an all-ones 128x128 stationary matmul accumulating over j; P^T @ V
    accumulates per head into outT (hd, n-cols) the same way. outT scaled by
    1/l (DVE reciprocal + multiply) once per (head, chunk).
  - Per (head, chunk) AllGather (groups of 4 cores) of outT in bf16.
  - Output projection y^T[n-slice] = woT^T @ gathered (bf16) + bias, emitted
    after the main loop; the Tile scheduler overlaps it with later chunks
    and it hides the last gather's DRAM round-trip latency.

PSUM banks: 4 rotating (128,512)-f32 shared by QKV/scores/projection, 2 PV
accumulators, 1 l accumulator, 1 bf16 V-transpose staging = 8 banks (16KB).
"""

import math
from contextlib import ExitStack

import numpy as np
import ml_dtypes

BF16_NP = ml_dtypes.bfloat16

import concourse.mybir as mybir
import concourse.tile as tile
from concourse import bacc
from concourse.bass_utils import run_bass_kernel_spmd
from concourse.masks import make_identity

FP32 = mybir.dt.float32
BF16 = mybir.dt.bfloat16

N_CORES = 8
TP = 4  # tensor-parallel group size (heads)
HPC = 4  # heads per core
B, S, D = 2, 2048, 2048
HD = 128
C_SCALE = 1.0 / math.sqrt(HD)
RG = [[0, 1, 2, 3], [4, 5, 6, 7]]

_NC_CACHE = {}


def build_nc(reps: int = 1, fake_collective: bool = False):
    key = (reps, fake_collective)
    if key in _NC_CACHE:
        return _NC_CACHE[key]
    nc = bacc.Bacc("TRN2", target_bir_lowering=False, debug=False, num_devices=N_CORES)

    xT_d = nc.declare_dram_parameter("xT", [D, S], BF16, isOutput=False)
    wqkT_d = nc.declare_dram_parameter("wqkT", [D, 2 * HPC * HD], BF16, isOutput=False)
    wvT_d = nc.declare_dram_parameter("wvT", [D, HPC * HD], BF16, isOutput=False)
    bqk_d = nc.declare_dram_parameter("bqk", [128, 2 * HPC], FP32, isOutput=False)
    bv_d = nc.declare_dram_parameter("bv", [128, HPC], FP32, isOutput=False)
    woT_d = nc.declare_dram_parameter("woT", [D, HPC * HD], BF16, isOutput=False)
    bo_d = nc.declare_dram_parameter("bo", [128, HPC], FP32, isOutput=False)
    y_t_d = nc.declare_dram_parameter("y_t", [HPC * HD, S], FP32, isOutput=True)

    with tile.TileContext(nc, num_cores=N_CORES) as tc, ExitStack() as octx:
        cpool = octx.enter_context(tc.tile_pool(name="const", bufs=1))
        ident = cpool.tile([128, 128], BF16, tag="ident", name="ident")
        make_identity(nc, ident[:])
        ones = cpool.tile([128, 128], BF16, tag="ones", name="ones")
        nc.gpsimd.memset(ones[:], 1.0)
        bqk_sb = cpool.tile([128, 2 * HPC], FP32, tag="bqk", name="bqk")
        nc.sync.dma_start(out=bqk_sb[:], in_=bqk_d[:])
        bv_sb = cpool.tile([128, HPC], FP32, tag="bv", name="bv")
        nc.sync.dma_start(out=bv_sb[:], in_=bv_d[:])
        bo_sb = cpool.tile([128, HPC], FP32, tag="bo", name="bo")
        nc.sync.dma_start(out=bo_sb[:], in_=bo_d[:])

        for rep in range(reps):
            sfx = f"r{rep}"
            # per (head, seq-quarter) gather tensors, bf16
            cc_in = [[nc.dram_tensor(f"cc_in{h}_{s}_{sfx}", [HD, S // 4], BF16)
                      for s in range(4)] for h in range(HPC)]
            cc_out = [[nc.dram_tensor(f"cc_out{h}_{s}_{sfx}", [TP * HD, S // 4], BF16)
                       for s in range(4)] for h in range(HPC)]
            _body(nc, tc, xT_d, wqkT_d, wvT_d, woT_d, y_t_d,
                  bqk_sb, bv_sb, bo_sb, ident, ones, cc_in, cc_out,
                  fake_collective)

    nc.compile()
    _NC_CACHE[key] = nc
    return nc


def _gather(nc, cc_in_t, cc_out_t, src_ap, fake):
    nc.sync.dma_start(out=cc_in_t[:], in_=src_ap)
    if fake:
        # timing stand-in only (wrong numerics): model a free/fully-
        # overlapped collective with realistic local DMA traffic
        nc.sync.dma_start(out=cc_out_t[:HD, :], in_=cc_in_t[:])
    else:
        nc.gpsimd.collective_compute(
            "AllGather", mybir.AluOpType.bypass, replica_groups=RG,
            ins=[cc_in_t[:]], outs=[cc_out_t[:]])


def _body(nc, tc, xT_d, wqkT_d, wvT_d, woT_d, y_t_d,
          bqk_sb, bv_sb, bo_sb, ident, ones, cc_in, cc_out,
          fake_collective=False):
    with ExitStack() as ctx:
        qkv_pool = ctx.enter_context(tc.tile_pool(name="qkv", bufs=1))
        # qT/kT per local head: (hd=128, S) bf16;  m 0-3 = q heads, 4-7 = k heads
        qkT_sb = [qkv_pool.tile([128, S], BF16, tag=f"qk{m}", name=f"qk{m}")
                  for m in range(8)]
        # v per local head: (seq-within-block=128, 16 blocks * 128) bf16
        vh_sb = [qkv_pool.tile([128, S], BF16, tag=f"vh{h}", name=f"vh{h}")
                 for h in range(HPC)]

        # weights live in single wide tiles, kc blocks side by side in the
        # free dim, so each loads with 4 grouped DMAs instead of 16
        wA = ctx.enter_context(tc.tile_pool(name="wA", bufs=1))
        wqk_sb = wA.tile([128, 16 * 1024], BF16, tag="wqk", name="wqk")
        wv_sb = wA.tile([128, 16 * 512], BF16, tag="wv", name="wv")
        wo_sb = wA.tile([128, 16 * 512], BF16, tag="wo", name="wo")

        xpool = ctx.enter_context(tc.tile_pool(name="xA", bufs=8))
        vtpool = ctx.enter_context(tc.tile_pool(name="vt", bufs=3))
        ptpool = ctx.enter_context(tc.tile_pool(name="pt", bufs=8))
        rpool = ctx.enter_context(tc.tile_pool(name="rv", bufs=2))
        otpool = ctx.enter_context(tc.tile_pool(name="ot", bufs=8))
        gpool = ctx.enter_context(tc.tile_pool(name="gD", bufs=5))
        ypool = ctx.enter_context(tc.tile_pool(name="yD", bufs=2))

        psW = ctx.enter_context(tc.tile_pool(name="psW", bufs=4, space="PSUM"))
        psPV = ctx.enter_context(tc.tile_pool(name="psPV", bufs=2, space="PSUM"))
        psLB = ctx.enter_context(tc.tile_pool(name="psLB", bufs=1, space="PSUM"))
        psTP = ctx.enter_context(tc.tile_pool(name="psTP", bufs=1, space="PSUM"))

        xT_r = xT_d.rearrange("(kc p) f -> p kc f", p=128)
        wqk_r = wqkT_d.rearrange("(kc p) f -> p kc f", p=128)
        wv_r = wvT_d.rearrange("(kc p) f -> p kc f", p=128)
        wo_r = woT_d.rearrange("(kc p) f -> p kc f", p=128)

        def load_x(n, g0=0, g1=4):
            xgs = []
            for g in range(g0, g1):
                xg = xpool.tile([128, 4 * 512], BF16, tag="xt", name="xt")
                nc.sync.dma_start(
                    out=xg[:].rearrange("p (kc f) -> p kc f", kc=4),
                    in_=xT_r[:, g * 4:(g + 1) * 4, n * 512:(n + 1) * 512])
                xgs.append(xg)
            return xgs

        # startup: interleave qk-weight loads with chunk 0's x so the first
        # accumulation chain is fed as early as possible (first group split
        # in two so the very first matmul starts sooner)
        for g2 in range(2):
            nc.sync.dma_start(
                out=wqk_sb[:, g2 * 2048:(g2 + 1) * 2048].rearrange(
                    "p (kc f) -> p kc f", kc=2),
                in_=wqk_r[:, g2 * 2:(g2 + 1) * 2, :])
        xgs_cur = load_x(0)

        for g in range(1, 4):
            nc.sync.dma_start(
                out=wqk_sb[:, g * 4096:(g + 1) * 4096].rearrange(
                    "p (kc f) -> p kc f", kc=4),
                in_=wqk_r[:, g * 4:(g + 1) * 4, :])
        for g in range(4):
            nc.sync.dma_start(
                out=wv_sb[:, g * 2048:(g + 1) * 2048].rearrange(
                    "p (kc f) -> p kc f", kc=4),
                in_=wv_r[:, g * 4:(g + 1) * 4, :])

        for n in range(4):  # seq chunks of 512
            if n == 1:
                # wo is first needed when chunk 0's gather lands; deferring
                # its load keeps startup DMA bandwidth for x and qkv weights
                for g in range(4):
                    nc.sync.dma_start(
                        out=wo_sb[:, g * 2048:(g + 1) * 2048].rearrange(
                            "p (kc f) -> p kc f", kc=4),
                        in_=wo_r[:, g * 4:(g + 1) * 4, :])
            ncol = slice(n * 512, (n + 1) * 512)
            xts = [xgs_cur[kc // 4][:, (kc % 4) * 512:(kc % 4 + 1) * 512]
                   for kc in range(16)]

            # head-major order for n>0: finish head h's q (m=h), k (m=4+h),
            # v (m=8+h) back to back so attention for head h starts while
            # later heads' projections still run. Chunk 0's attention is tiny
            # and its v weights arrive late, so keep sequential order there.
            m_order = (range(12) if n == 0
                       else [0, 4, 8, 1, 5, 9, 2, 6, 10, 3, 7, 11])
            for m in m_order:
                psm = psW.tile([128, 512], FP32, tag="w512", name="psA")
                for kc in range(16):
                    if m < 8:
                        o = kc * 1024 + m * 128
                        lhsT = wqk_sb[:, o:o + 128]
                    else:
                        o = kc * 512 + (m - 8) * 128
                        lhsT = wv_sb[:, o:o + 128]
                    nc.tensor.matmul(psm[:], lhsT, xts[kc][:],
                                     start=(kc == 0), stop=(kc == 15))
                if m < 8:
                    nc.vector.tensor_scalar_add(
                        qkT_sb[m][:, ncol], psm[:], bqk_sb[:, m:m + 1])
                else:
                    h = m - 8
                    vt = vtpool.tile([128, 512], BF16, tag="vt", name="vt")
                    nc.vector.tensor_scalar_add(
                        vt[:], psm[:], bv_sb[:, h:h + 1])
                    tps = psTP.tile([128, 512], BF16, tag="tp2", name="tp2")
                    for j in range(4):
                        nc.tensor.transpose(
                            tps[:, j * 128:(j + 1) * 128],
                            vt[:, j * 128:(j + 1) * 128], ident[:])
                    nc.vector.tensor_copy(vh_sb[h][:, ncol], tps[:])

            if n < 3:  # prefetch next chunk's x ahead of the attention DMAs
                xgs_cur = load_x(n + 1)

            nj = 4 * n + 4
            for h in range(HPC):
                lb = psLB.tile([128, 512], FP32, tag="lb", name="lb")
                pv = psPV.tile([128, 512], FP32, tag="pv", name="pv")
                for j in range(nj):
                    diag = j >= 4 * n
                    c0 = j * 128 if diag else n * 512
                    w = (n + 1) * 512 - c0
                    off = c0 - n * 512
                    St = psW.tile([128, w], FP32, tag="w512", name="St",
                                  padded_shape=[128, 512])
                    nc.tensor.matmul(
                        St[:], qkT_sb[HPC + h][:, j * 128:(j + 1) * 128],
                        qkT_sb[h][:, c0:(n + 1) * 512],
                        start=True, stop=True, skip_group_check=True)
                    pt = ptpool.tile([128, 512], BF16, tag="pt", name="pt")
                    nc.scalar.activation(
                        pt[:, :w], St[:],
                        mybir.ActivationFunctionType.Exp,
                        bias=0.0, scale=C_SCALE)
                    if diag:
                        # zero the strictly-lower triangle (q < k) in place
                        nc.gpsimd.affine_select(
                            out=pt[:, :128], in_=pt[:, :128],
                            pattern=[[1, 128]],
                            compare_op=mybir.AluOpType.is_ge, fill=0.0,
                            base=0, channel_multiplier=-1)
                    nc.tensor.matmul(
                        lb[:, off:off + w], ones[:], pt[:, :w],
                        start=(j == 0), stop=(j == nj - 1),
                        skip_group_check=True)
                    nc.tensor.matmul(
                        pv[:, off:off + w],
                        vh_sb[h][:, j * 128:(j + 1) * 128], pt[:, :w],
                        start=(j == 0), stop=(j == nj - 1),
                        skip_group_check=True)
                rinv = rpool.tile([128, 512], FP32, tag="rinv", name="rinv")
                nc.vector.reciprocal(rinv[:], lb[:])
                ot = otpool.tile([128, 512], BF16, tag="ot", name="ot")
                nc.vector.tensor_mul(ot[:], pv[:], rinv[:])
                _gather(nc, cc_in[h][n], cc_out[h][n], ot[:], fake_collective)

        # ---- output projection (scheduler overlaps with later chunks) ----
        with nc.named_scope("out_proj"):
            for n in range(4):
                ncol_out = slice(n * 512, (n + 1) * 512)
                gts = []
                for h in range(HPC):
                    gtb = gpool.tile([128, 4 * 512], BF16, tag="gt", name="gt")
                    nc.sync.dma_start(
                        out=gtb[:].rearrange("p (r f) -> p r f", r=4),
                        in_=cc_out[h][n].rearrange("(r p) f -> p r f", p=128))
                    gts.extend(gtb[:, r * 512:(r + 1) * 512] for r in range(4))
                for m in range(4):
                    psy = psW.tile([128, 512], FP32, tag="w512", name="py")
                    for kc in range(16):
                        o = kc * 512 + m * 128
                        nc.tensor.matmul(
                            psy[:], wo_sb[:, o:o + 128],
                            gts[kc][:], start=(kc == 0), stop=(kc == 15))
                    yt = ypool.tile([128, 512], FP32, tag="yt", name="yt")
                    nc.scalar.activation(
                        yt[:], psy[:],
                        mybir.ActivationFunctionType.Identity,
                        bias=bo_sb[:, m:m + 1], scale=1.0)
                    nc.sync.dma_start(
                        out=y_t_d[m * 128:(m + 1) * 128, ncol_out], in_=yt[:])


def make_in_maps(x, w_qkv, b_qkv, w_out, b_out):
    in_maps = []
    # gathered row g = h*512 + r*128 + i  <->  w_out column (4r+h)*128 + i
    dorder = np.array(
        [(4 * r + h) * 128 + i for h in range(HPC) for r in range(TP)
         for i in range(HD)])
    for c in range(N_CORES):
        b, t = divmod(c, TP)
        xT = np.ascontiguousarray(x[b].T)
        wq = w_qkv[512 * t:512 * (t + 1)]
        wk = w_qkv[D + 512 * t:D + 512 * (t + 1)]
        wv = w_qkv[2 * D + 512 * t:2 * D + 512 * (t + 1)]
        wqkT = np.ascontiguousarray(np.concatenate([wq, wk], axis=0).T)
        wvT = np.ascontiguousarray(wv.T)
        offs_qk = [512 * t + hh * 128 for hh in range(4)] + \
                  [D + 512 * t + hh * 128 for hh in range(4)]
        bqk = np.stack([b_qkv[o:o + 128] for o in offs_qk], axis=1)
        bv = np.stack(
            [b_qkv[2 * D + 512 * t + hh * 128:2 * D + 512 * t + hh * 128 + 128]
             for hh in range(4)], axis=1)
        woT = np.ascontiguousarray(w_out[512 * t:512 * (t + 1)][:, dorder].T)
        bo = np.ascontiguousarray(b_out[512 * t:512 * (t + 1)].reshape(4, 128).T)
        in_maps.append(dict(
            xT=xT.astype(BF16_NP), wqkT=wqkT.astype(BF16_NP),
            wvT=wvT.astype(BF16_NP),
            bqk=np.ascontiguousarray(bqk), bv=np.ascontiguousarray(bv),
            woT=woT.astype(BF16_NP), bo=bo))
    return in_maps


def assemble_y(results):
    y = np.empty((B, S, D), np.float32)
    for c in range(N_CORES):
        b, t = divmod(c, TP)
        y[b][:, 512 * t:512 * (t + 1)] = results[c]["y_t"].T
    return y


def kernel(x, w_qkv, b_qkv, w_out, b_out):
    x = np.asarray(x, dtype=np.float32)
    w_qkv = np.asarray(w_qkv, dtype=np.float32)
    b_qkv = np.asarray(b_qkv, dtype=np.float32)
    w_out = np.asarray(w_out, dtype=np.float32)
    b_out = np.asarray(b_out, dtype=np.float32)

    nc = build_nc(1)
    in_maps = make_in_maps(x, w_qkv, b_qkv, w_out, b_out)
    r = run_bass_kernel_spmd(nc, in_maps, list(range(N_CORES)))
    return assemble_y(r.results)
